# revision 2
# baseline (speedup 1.0000x reference)
"""Inverse discrete Hough transform on 8 Trainium2 NeuronCores — v2.

out[n, c, y, x] = sum_a acc[n, c, a, r(a, y, x)],
r(a, y, x) = round(x' cos_a + y' sin_a) + R/2  (static index table).

v3 = v2 + bit-packed one-hots: the fp8 one-hot stationaries (30.4MB
of HBM traffic in v2) ship as 1 bit/entry (3.8MB) and are expanded
on-device by DVE/GpSimd shift-and ops into fp8 weights with value
2^-6 (bit pattern 0x08); the missing x64 is folded into the
psum->SBUF output copies. Packing: one-hot column p (pixel) lives in
packed byte j = p % 16, bit b = p // 16, so expansion for bit b
writes the contiguous fp8 slice [b*16:(b+1)*16].

v2 strategy (vs the v1 per-128px-tile scheme): pixel-shard by y (each
core owns 32 output rows, all 256 channels). Pixels are grouped into
16x16 SUPER-tiles (256 px); each super splits into two 128-px psum
sub-tiles (left/right 16x8). Per super, the per-angle rho index ranges
("bands") over the whole super are concatenated into P=29 passes of
K=128 rows. Each pass streams ONE moving tile (the gathered acc rows,
bf16 [128 x 256ch]) through TWO matmuls whose stationaries are the
fp8e4 one-hot maps of the two sub-tiles. Sharing the moving stream
between 2 psum tiles nearly halves the gathered-row HBM traffic vs v1
(60.8MB vs 96.5/193MB), and fp8 one-hots halve that stream too
(30.4MB vs 48MB bf16). Output is written bf16 (4.2MB) and upcast on
host. HBM per core ~95MB vs ~240MB for v1.
"""
import sys, os

sys.path.insert(0, "/opt/trn_rl_repo")
import numpy as np
import ml_dtypes

from concourse import bass, tile
from concourse.bass_utils import run_bass_kernel_spmd
import concourse.mybir as mybir

# ---------------- problem constants (hardcoded) ----------------
OUT_H = 256
OUT_W = 256
NUMANGLE = 180
NUMRHO = 400
N_B, C_CH = 4, 64
NCH = N_B * C_CH  # 256 channels
N_CORES = 8
ROWS_PER_CORE = OUT_H // N_CORES  # 32 y-rows per core
SH, SW = 16, 16  # super-tile: 16y x 16x = 256 px -> 2 psum sub-tiles
NSUP_Y = ROWS_PER_CORE // SH  # 2
NSUP_X = OUT_W // SW  # 16
NSUP = NSUP_Y * NSUP_X  # 32
SUB_PX = 128  # pixels per psum sub-tile (left 16x8 / right 16x8)
P = 29  # passes per super (uniform across cores/supers, verified)

f32 = mybir.dt.float32
bf16 = mybir.dt.bfloat16
fp8 = mybir.dt.float8e4
u8 = mybir.dt.uint8
u16 = mybir.dt.uint16

_MAX_INSTR_WAITS = 1


def _split_excess_waits(nc):
    """walrus's TRN2 codegen allows only one sync-wait command on several
    instruction structs. Move excess waits onto injected same-engine NoOps
    placed just before the over-subscribed instruction."""
    n = 0
    for fn in nc.m.functions:
        for bb in fn.blocks:
            out = []
            changed = False
            for inst in bb.instructions:
                si = inst.sync_info
                waits = list(si.on_wait) if si and si.on_wait else []
                if len(waits) > _MAX_INSTR_WAITS:
                    for w in waits[_MAX_INSTR_WAITS:]:
                        nop = mybir.InstNoOp(
                            name=f"waitsplit-{n}-{inst.name}", ins=[], outs=[]
                        )
                        n += 1
                        nop.engine = inst.engine
                        nop.sync_info = mybir.SyncInfo(on_wait=[w], on_update=[])
                        out.append(nop)
                    inst.sync_info = mybir.SyncInfo(
                        on_wait=waits[:_MAX_INSTR_WAITS],
                        on_update=list(si.on_update or []),
                    )
                    changed = True
                out.append(inst)
            if changed:
                bb.instructions = out
    return n


def _install_ntff_hook():
    try:
        import types
        import antenv

        if hasattr(antenv, "axon_hooks"):
            return
        from trn_agent_boot.trn_boot import _ntff_profile_via_ctypes

        hook = _ntff_profile_via_ctypes("/opt/axon/libaxon_pjrt.so")
        mod = types.ModuleType("antenv.axon_hooks")
        mod.get_axon_ntff_profile_hook = lambda: hook
        mod.set_axon_ntff_profile_hook = lambda h: None
        sys.modules["antenv.axon_hooks"] = mod
        antenv.axon_hooks = mod
    except Exception:
        pass


_install_ntff_hook()


# ---------------- static index tables ----------------
def _rho_index_table():
    """Mirror of the reference's jnp fp32 math (through jax so rounding
    matches the harness's reference bit-for-bit)."""
    try:
        import jax
        import jax.numpy as jnp

        with jax.default_device(jax.devices("cpu")[0]):
            angles = jnp.arange(NUMANGLE, dtype=jnp.float32) * (np.pi / NUMANGLE)
            cos_t = jnp.cos(angles)
            sin_t = jnp.sin(angles)
            xs = (jnp.arange(OUT_W) - OUT_W // 2).astype(jnp.float32)
            ys = (jnp.arange(OUT_H) - OUT_H // 2).astype(jnp.float32)
            r = jnp.round(
                xs[None, None, :] * cos_t[:, None, None]
                + ys[None, :, None] * sin_t[:, None, None]
            ).astype(jnp.int32) + NUMRHO // 2
            r = jnp.clip(r, 0, NUMRHO - 1)
            return np.asarray(r)
    except Exception:
        angles = (
            np.arange(NUMANGLE, dtype=np.float32) * np.float32(np.pi / NUMANGLE)
        ).astype(np.float32)
        cos_t = np.cos(angles).astype(np.float32)
        sin_t = np.sin(angles).astype(np.float32)
        xs = (np.arange(OUT_W) - OUT_W // 2).astype(np.float32)
        ys = (np.arange(OUT_H) - OUT_H // 2).astype(np.float32)
        z = (
            xs[None, None, :] * cos_t[:, None, None]
            + ys[None, :, None] * sin_t[:, None, None]
        )
        r = np.round(z).astype(np.int32) + NUMRHO // 2
        return np.clip(r, 0, NUMRHO - 1)


_STATIC = {}


def _build_static():
    """Per-core moving-row gather indices + fp8 one-hot tables.

    rowidx[core]: [128, NSUP*P] int64 flat accT row ids (partition-major
        to match the SBUF tile layout [128, P, NCH] per super).
    oh[core]:     [128, NSUP*P, 2, 128] fp8  one-hot stationaries.
    Pixel order inside sub-tile s of super (sy, sx):
        px = yl * (SW//2) + xl  over  y = 32*core + sy*SH + yl,
        x = sx*SW + s*(SW//2) + xl.
    """
    if _STATIC:
        return _STATIC
    r_idx = _rho_index_table()  # [A, H, W]

    per_core_rows = []
    per_core_oh = []
    for core in range(N_CORES):
        y0 = core * ROWS_PER_CORE
        rowidx = np.zeros((NSUP * P, 128), np.int64)
        oh = np.zeros((NSUP * P * 128, 2, SUB_PX), np.float32)
        for s in range(NSUP):
            sy, sx = divmod(s, NSUP_X)
            rs = r_idx[
                :, y0 + sy * SH : y0 + (sy + 1) * SH, sx * SW : (sx + 1) * SW
            ]  # [A, SH, SW]
            # sub-tile pixel tables [A, 2, 128]
            rsub = np.stack(
                [
                    rs[:, :, : SW // 2].reshape(NUMANGLE, SUB_PX),
                    rs[:, :, SW // 2 :].reshape(NUMANGLE, SUB_PX),
                ],
                axis=1,
            )
            flat = rs.reshape(NUMANGLE, -1)
            lo = flat.min(axis=1)
            hi = flat.max(axis=1)
            widths = hi - lo + 1
            L = int(widths.sum())
            assert L <= P * 128, (core, s, L)
            a_arr = np.repeat(np.arange(NUMANGLE), widths)
            rho_arr = np.concatenate(
                [np.arange(lo[a], hi[a] + 1) for a in range(NUMANGLE)]
            )
            base = s * P * 128
            rowidx.reshape(-1)[base : base + L] = (
                a_arr.astype(np.int64) * NUMRHO + rho_arr
            )
            # one-hot: row k selects pixels px with r(a_k, sub, px) == rho_k
            oh[base : base + L] = (
                rsub[a_arr] == rho_arr[:, None, None]
            )
            # padding rows keep rowidx 0 / oh 0.
        # device layouts: rowidx [NSUP*P, 128] -> [128, NSUP*P]
        per_core_rows.append(np.ascontiguousarray(rowidx.T))
        # bit-pack: one-hot column p -> byte j = p % 16, bit b = p // 16
        ohb = oh.reshape(NSUP * P, 128, 2, 8, 16).astype(np.uint8)
        packed = np.zeros((NSUP * P, 128, 2, 16), np.uint8)
        for b in range(8):
            packed |= ohb[:, :, :, b, :] << b
        per_core_oh.append(
            np.ascontiguousarray(packed.transpose(1, 0, 2, 3))
        )  # [128, NSUP*P, 2, 16] uint8

    _STATIC["rowidx"] = per_core_rows
    _STATIC["oh"] = per_core_oh
    return _STATIC


# ---------------- device program ----------------
_PROGRAM = {}


def _build_program():
    if "nc" in _PROGRAM:
        return _PROGRAM["nc"]
    nc = bass.Bass()
    mov_dram = nc.declare_dram_parameter(
        "mov", [128, NSUP * P, NCH], bf16, isOutput=False
    )
    oh_dram = nc.declare_dram_parameter(
        "oh", [128, NSUP * P, 2, 16], u8, isOutput=False
    )
    out_dram = nc.declare_dram_parameter(
        "out", [NSUP, 128, 2, NCH], bf16, isOutput=True
    )

    with tile.TileContext(nc) as tc:
        with (
            tc.tile_pool(name="mov", bufs=4) as movp,
            tc.tile_pool(name="oh", bufs=4) as ohp,
            tc.tile_pool(name="out", bufs=2) as outp,
            tc.tile_pool(name="psum", bufs=3, space="PSUM") as psump,
        ):
            for s in range(NSUP):
                mov_sb = movp.tile([128, P, NCH], bf16)
                # split the dominant mov stream across two DMA queues
                half = P // 2
                nc.sync.dma_start(
                    mov_sb[:, :half, :], mov_dram[:, s * P : s * P + half, :]
                )
                nc.gpsimd.dma_start(
                    mov_sb[:, half:, :], mov_dram[:, s * P + half : (s + 1) * P, :]
                )
                ohb_sb = ohp.tile([128, P, 2, 16], u8)
                nc.scalar.dma_start(ohb_sb[:], oh_dram[:, s * P : (s + 1) * P, :, :])
                exp_sb = ohp.tile([128, P, 2, SUB_PX], fp8)
                # expand bit b of packed byte j to fp8 0x08 (=2^-6) at
                # one-hot column p = b*16 + j; compensated by x64 in the
                # psum->SBUF copies below. Processed as uint16 pairs (2x
                # DVE rate); the per-byte shift-and works under a 0x0808
                # mask since cross-byte spill bits land outside it.
                ohb16 = ohb_sb[:].bitcast(u16)
                for b in range(8):
                    dst = exp_sb[:, :, :, b * 16 : (b + 1) * 16].bitcast(u16)
                    if b <= 3:
                        nc.vector.tensor_scalar(
                            dst, ohb16, 3 - b, 0x0808,
                            mybir.AluOpType.logical_shift_left,
                            mybir.AluOpType.bitwise_and,
                        )
                    else:
                        nc.vector.tensor_scalar(
                            dst, ohb16, b - 3, 0x0808,
                            mybir.AluOpType.logical_shift_right,
                            mybir.AluOpType.bitwise_and,
                        )
                ps_a = psump.tile([128, NCH], f32)
                ps_b = psump.tile([128, NCH], f32)
                for p in range(P):
                    nc.tensor.matmul(
                        ps_a[:],
                        exp_sb[:, p, 0, :],
                        mov_sb[:, p, :],
                        start=(p == 0),
                        stop=(p == P - 1),
                    )
                    nc.tensor.matmul(
                        ps_b[:],
                        exp_sb[:, p, 1, :],
                        mov_sb[:, p, :],
                        start=(p == 0),
                        stop=(p == P - 1),
                    )
                y = outp.tile([128, 2, NCH], bf16)
                nc.scalar.activation(
                    y[:, 0, :], ps_a[:],
                    mybir.ActivationFunctionType.Copy, scale=64.0,
                )
                nc.scalar.activation(
                    y[:, 1, :], ps_b[:],
                    mybir.ActivationFunctionType.Copy, scale=64.0,
                )
                nc.scalar.dma_start(out_dram[s], y[:])

    _split_excess_waits(nc)
    _PROGRAM["nc"] = nc
    return nc


# ---------------- entry point ----------------
def _run(accumulator: np.ndarray, trace: bool = False):
    st = _build_static()
    nc = _build_program()

    accT = np.ascontiguousarray(
        accumulator.transpose(2, 3, 0, 1)
    ).reshape(NUMANGLE * NUMRHO, NCH).astype(ml_dtypes.bfloat16)

    in_maps = []
    for core in range(N_CORES):
        mov = accT[st["rowidx"][core]]  # [128, NSUP*P, 256] bf16
        in_maps.append({"mov": mov, "oh": st["oh"][core]})

    res = run_bass_kernel_spmd(nc, in_maps, list(range(N_CORES)), trace=trace)

    # reassemble: out_c [NSUP, 2, 128, NCH] -> [NCH, 32y, 256x]
    parts = []
    for core in range(N_CORES):
        oc = np.asarray(res.results[core]["out"]).astype(np.float32)
        # [NSUP_Y, NSUP_X, yl, xl, sub, NCH]
        oc = oc.reshape(NSUP_Y, NSUP_X, SH, SW // 2, 2, NCH)
        oc = oc.transpose(5, 0, 2, 1, 4, 3)  # [NCH, sy, yl, sx, sub, xl]
        parts.append(oc.reshape(NCH, ROWS_PER_CORE, OUT_W))
    full = np.concatenate(parts, axis=1)  # [256, 256, 256]
    out = full.reshape(N_B, C_CH, OUT_H, OUT_W)
    return out, res


def kernel(accumulator: np.ndarray) -> np.ndarray:
    out, _ = _run(np.asarray(accumulator, dtype=np.float32), trace=False)
    return out
